# revision 18
# baseline (speedup 1.0000x reference)
"""DTW kernel for nn_DTW_56272661512310 — Bass/Tile implementation.

Sharding (per hint): data-parallel over batch B=64 across 8 NeuronCores
(8 samples per core); scalars a, b replicated.  One SPMD Bass program is
compiled once and run on cores 0-7 via run_bass_kernel_spmd.

Per-core pipeline:
  1. Load e1/e2 natural-layout fp32 tiles [128 rows, 768], compute row
     norms (ACT square+accum -> sqrt -> DVE reciprocal), normalize+cast
     to bf16 (DVE tensor_scalar with per-partition 1/norm).
  2. DMA-xbar transpose bf16 tiles SBUF->SBUF into [D, rows] layout.
  3. PE: sim = n1 @ n2^T per sample (6 K-tiles accumulate in PSUM).
  4. ACT: T = tanh(a*sim + b) (scale/bias are per-partition APs
     broadcast from the runtime a,b inputs).  DVE: relu for cols>=1
     (col 0 stays raw — it feeds the DP max-only update).  Rows are
     written to a DRAM scratch in DP layout [I, samples, J].
  5. DTW DP on DVE: row-scan reformulation
         M[i,j] = max(M[i-1,j], M[i-1,j-1] + Rp[i,j])
     with M kept as [8 samples, 1+J] ("guard zero" in col 0 makes the
     shifted read a single AP).  Rp rows stream back from DRAM in
     chunks.  Answer = (M[J-2] + relu(T[I-1,J-1])) / J.
"""

import numpy as np

N_CORES = 8
B, I, J, D = 64, 512, 384, 768
BP = B // N_CORES  # samples per core


def build_nc(bp=BP, i_dim=I, j_dim=J, d_dim=D, chunk=16):
    import concourse.bass as bass
    import concourse.mybir as mybir
    import concourse.tile as tile
    from concourse import bacc
    from contextlib import ExitStack

    f32 = mybir.dt.float32
    bf16 = mybir.dt.bfloat16
    P = 128
    IT = i_dim // P  # i row-tiles
    JT = (j_dim + P - 1) // P  # j row-tiles (for e2)
    KT = d_dim // P  # contraction tiles

    nc = bacc.Bacc("TRN2", target_bir_lowering=False)
    e1 = nc.dram_tensor("e1", [bp, i_dim, d_dim], f32, kind="ExternalInput")
    e2 = nc.dram_tensor("e2", [bp, j_dim, d_dim], f32, kind="ExternalInput")
    a_in = nc.dram_tensor("a", [1], f32, kind="ExternalInput")
    b_in = nc.dram_tensor("b", [1], f32, kind="ExternalInput")
    out = nc.dram_tensor("out", [bp], f32, kind="ExternalOutput")
    dp_r = nc.dram_tensor("dp_r", [i_dim, bp, j_dim], f32)

    with tile.TileContext(nc) as tc, ExitStack() as ctx:
        const_pool = ctx.enter_context(tc.tile_pool(name="const", bufs=1))
        dp_pool = ctx.enter_context(tc.tile_pool(name="dp", bufs=1))
        sctx = ExitStack()
        nat_pool = sctx.enter_context(tc.tile_pool(name="nat", bufs=3))
        sq_pool = sctx.enter_context(tc.tile_pool(name="sq", bufs=2))
        nrm_pool = sctx.enter_context(tc.tile_pool(name="nrm", bufs=4))
        bfn_pool = sctx.enter_context(tc.tile_pool(name="bfn", bufs=3))
        e1t_pool = sctx.enter_context(tc.tile_pool(name="e1t", bufs=bp))
        e2t_pool = sctx.enter_context(tc.tile_pool(name="e2t", bufs=bp))
        psum_pool = sctx.enter_context(tc.tile_pool(name="ps", bufs=4, space="PSUM"))
        tanh_pool = sctx.enter_context(tc.tile_pool(name="tnh", bufs=3))
        relu_pool = sctx.enter_context(tc.tile_pool(name="rlu", bufs=3))

        def absorb(eng, *prods, hint="absorb"):
            # Park multi-source sem waits on a NOP (which has room for them)
            # so the following fragile-struct instruction needs none.
            no = eng.nop(nofuse=True, hint=hint)
            for p in prods:
                if p is not None:
                    add_dep_helper(
                        no.ins, p.ins if hasattr(p, "ins") else p,
                        reason="wait-absorb",
                    )
            return no

        # a, b -> per-partition scale/bias APs (DMA broadcast via step-0 AP)
        ab_bc = const_pool.tile([P, 2], f32)
        nc.sync.dma_start(ab_bc[:, 0:1], a_in[0:1].broadcast_to((P, 1)))
        nc.sync.dma_start(ab_bc[:, 1:2], b_in[0:1].broadcast_to((P, 1)))
        a_ap = ab_bc[:, 0:1]
        b_ap = ab_bc[:, 1:2]
        hist = {"load": [], "sq": []}

        def norm_cast_transpose(src, n_rows_tiles, n_rows, dst_tiles, s):
            # src: dram [n_rows, d_dim] for sample s; dst_tiles: per-k bf16
            # [P, n_rows] transposed tiles.
            for t in range(n_rows_tiles):
                r0 = t * P
                rows = min(P, n_rows - r0)
                x = nat_pool.tile([P, d_dim], f32, tag="nat")
                nc.sync.dma_start(x[:rows, :], src[r0 : r0 + rows, :])
                sq = sq_pool.tile([P, d_dim], f32, tag="sq")
                n2 = nrm_pool.tile([P, 1], f32, tag="n2")
                nc.scalar.activation(
                    sq[:rows, :],
                    x[:rows, :],
                    mybir.ActivationFunctionType.Square,
                    accum_out=n2[:rows, :],
                )
                nrm = nrm_pool.tile([P, 1], f32, tag="nrm")
                nc.scalar.sqrt(nrm[:rows, :], n2[:rows, :])
                inv = nrm_pool.tile([P, 1], f32, tag="inv")
                nc.vector.reciprocal(inv[:rows, :], nrm[:rows, :])
                xb = bfn_pool.tile([P, d_dim], bf16, tag="xb")
                nc.scalar.mul(xb[:rows, :], x[:rows, :], inv[:rows, :])
                # One xbar transpose for the whole [rows, d_dim] tile.  The
                # 3D out AP lands d = 6*p + k at (partition p, plane k) —
                # an interleaved-d layout; matmul works as long as lhsT and
                # rhs use the same (p, k) -> d mapping.
                nc.sync.dma_start_transpose(
                    dst_tiles[:, :, r0 : r0 + rows], xb[:rows, :]
                )

        for s in range(bp):
            e1t = e1t_pool.tile([P, KT, i_dim], bf16, tag="e1t", name=f"e1t_{s}")
            e2t = e2t_pool.tile([P, KT, j_dim], bf16, tag="e2t", name=f"e2t_{s}")
            norm_cast_transpose(e1[s], IT, i_dim, e1t, s)
            norm_cast_transpose(e2[s], JT, j_dim, e2t, s)

            for it in range(IT):
                ps = psum_pool.tile([P, j_dim], f32, tag="ps")
                for k in range(KT):
                    nc.tensor.matmul(
                        ps[:],
                        e1t[:, k, it * P : (it + 1) * P],
                        e2t[:, k, :],
                        start=(k == 0),
                        stop=(k == KT - 1),
                    )
                # relu(tanh(z)) == tanh(relu(z)); col 0 keeps the raw tanh.
                tr = tanh_pool.tile([P, j_dim], f32, tag="th")
                nc.scalar.activation(
                    tr[:, 1:],
                    ps[:, 1:],
                    mybir.ActivationFunctionType.Relu,
                    bias=b_ap,
                    scale=a_ap,
                )
                rp = relu_pool.tile([P, j_dim], f32, tag="rp")
                nc.scalar.activation(
                    rp[:, 1:], tr[:, 1:], mybir.ActivationFunctionType.Tanh
                )
                nc.scalar.activation(
                    rp[:, 0:1],
                    ps[:, 0:1],
                    mybir.ActivationFunctionType.Tanh,
                    bias=b_ap,
                    scale=a_ap,
                )
                nc.sync.dma_start(dp_r[it * P : (it + 1) * P, s, :], rp[:])

        # ---- DTW DP ----
        sctx.close()

        dpc_pool = ctx.enter_context(tc.tile_pool(name="dpc", bufs=2))
        dpt_pool = ctx.enter_context(tc.tile_pool(name="dpt", bufs=2))
        m_ext = dp_pool.tile([bp, j_dim + 1], f32, tag="m")
        nc.vector.memset(m_ext[:, 0:1], 0.0)
        nc.sync.dma_start(m_ext[:, 1:], dp_r[0, :, :])
        r_last = dp_pool.tile([bp, 1], f32, tag="rl")
        nc.sync.dma_start(r_last[:], dp_r[i_dim - 1 : i_dim, :, j_dim - 1])

        for c0 in range(1, i_dim - 1, chunk):
            ch = min(chunk, i_dim - 1 - c0)
            rch = dpc_pool.tile([bp, chunk, j_dim], f32, tag="rch")
            nc.sync.dma_start(
                rch[:, :ch, :], dp_r[c0 : c0 + ch, :, :].rearrange("i s j -> s i j")
            )
            for i in range(ch):
                tmp = dpt_pool.tile([bp, j_dim], f32, tag="tmp")
                nc.vector.tensor_add(tmp[:], m_ext[:, 0:j_dim], rch[:, i, :])
                nc.vector.tensor_max(m_ext[:, 1:], m_ext[:, 1:], tmp[:])

        ans = dp_pool.tile([bp, 1], f32, tag="ans")
        nc.vector.tensor_add(ans[:], m_ext[:, j_dim - 1 : j_dim], r_last[:])
        nc.vector.tensor_scalar_mul(ans[:], ans[:], 1.0 / j_dim)
        nc.sync.dma_start(out[:], ans[:, 0])

    return nc


_cache = {}


def _get_nc():
    if "nc" not in _cache:
        nc = build_nc()
        nc.finalize()  # run Bacc reg-alloc/DCE before the PJRT compile path
        _cache["nc"] = nc
    return _cache["nc"]


def make_in_maps(emb1, emb2, a, b):
    e1 = np.ascontiguousarray(np.asarray(emb1, np.float32).reshape(N_CORES, BP, I, D))
    e2 = np.ascontiguousarray(np.asarray(emb2, np.float32).reshape(N_CORES, BP, J, D))
    a32 = np.asarray(a, np.float32).reshape(1)
    b32 = np.asarray(b, np.float32).reshape(1)
    return [
        {"e1": e1[c], "e2": e2[c], "a": a32, "b": b32} for c in range(N_CORES)
    ]


def run_spmd(in_maps, **kwargs):
    from concourse.bass_utils import run_bass_kernel_spmd

    return run_bass_kernel_spmd(_get_nc(), in_maps, list(range(N_CORES)), **kwargs)


def kernel(emb1, emb2, a, b):
    res = run_spmd(make_in_maps(emb1, emb2, a, b))
    return np.concatenate(
        [np.asarray(res.results[c]["out"]) for c in range(N_CORES)]
    ).astype(np.float32)


if __name__ == "__main__":
    rng = np.random.default_rng(0)
    inputs = dict(
        emb1=rng.standard_normal((B, I, D), dtype=np.float32),
        emb2=rng.standard_normal((B, J, D), dtype=np.float32),
        a=rng.random((1,), dtype=np.float32),
        b=rng.random((1,), dtype=np.float32),
    )
    out = kernel(**inputs)
    print("out[:4]:", out[:4])


# revision 20
# speedup vs baseline: 1.1412x; 1.1412x over previous
"""DTW kernel for nn_DTW_56272661512310 — Bass/Tile implementation.

Sharding (per hint): data-parallel over batch B=64 across 8 NeuronCores
(8 samples per core); scalars a, b replicated.  One SPMD Bass program is
compiled once and run on cores 0-7 via run_bass_kernel_spmd.

Per-core pipeline:
  1. Load e1/e2 natural-layout fp32 tiles [128 rows, 768], compute row
     norms (ACT square+accum -> sqrt -> DVE reciprocal), normalize+cast
     to bf16 (DVE tensor_scalar with per-partition 1/norm).
  2. DMA-xbar transpose bf16 tiles SBUF->SBUF into [D, rows] layout.
  3. PE: sim = n1 @ n2^T per sample (6 K-tiles accumulate in PSUM).
  4. ACT: T = tanh(a*sim + b) (scale/bias are per-partition APs
     broadcast from the runtime a,b inputs).  DVE: relu for cols>=1
     (col 0 stays raw — it feeds the DP max-only update).  Rows are
     written to a DRAM scratch in DP layout [I, samples, J].
  5. DTW DP on DVE: row-scan reformulation
         M[i,j] = max(M[i-1,j], M[i-1,j-1] + Rp[i,j])
     with M kept as [8 samples, 1+J] ("guard zero" in col 0 makes the
     shifted read a single AP).  Rp rows stream back from DRAM in
     chunks.  Answer = (M[J-2] + relu(T[I-1,J-1])) / J.
"""

import numpy as np

N_CORES = 8
B, I, J, D = 64, 512, 384, 768
BP = B // N_CORES  # samples per core


def build_nc(bp=BP, i_dim=I, j_dim=J, d_dim=D, chunk=16):
    import concourse.bass as bass
    import concourse.mybir as mybir
    import concourse.tile as tile
    from concourse import bacc
    from contextlib import ExitStack

    f32 = mybir.dt.float32
    bf16 = mybir.dt.bfloat16
    P = 128
    IT = i_dim // P  # i row-tiles
    JT = (j_dim + P - 1) // P  # j row-tiles (for e2)
    KT = d_dim // P  # contraction tiles

    nc = bacc.Bacc("TRN2", target_bir_lowering=False)
    e1 = nc.dram_tensor("e1", [bp, i_dim, d_dim], f32, kind="ExternalInput")
    e2 = nc.dram_tensor("e2", [bp, j_dim, d_dim], f32, kind="ExternalInput")
    a_in = nc.dram_tensor("a", [1], f32, kind="ExternalInput")
    b_in = nc.dram_tensor("b", [1], f32, kind="ExternalInput")
    out = nc.dram_tensor("out", [bp], f32, kind="ExternalOutput")
    dp_r = nc.dram_tensor("dp_r", [i_dim, bp, j_dim], f32)

    with tile.TileContext(nc) as tc, ExitStack() as ctx:
        const_pool = ctx.enter_context(tc.tile_pool(name="const", bufs=1))
        dp_pool = ctx.enter_context(tc.tile_pool(name="dp", bufs=1))
        sctx = ExitStack()
        nat_pool = sctx.enter_context(tc.tile_pool(name="nat", bufs=3))
        sq_pool = sctx.enter_context(tc.tile_pool(name="sq", bufs=2))
        nrm_pool = sctx.enter_context(tc.tile_pool(name="nrm", bufs=4))
        bfn_pool = sctx.enter_context(tc.tile_pool(name="bfn", bufs=3))
        e1t_pool = sctx.enter_context(tc.tile_pool(name="e1t", bufs=bp))
        e2t_pool = sctx.enter_context(tc.tile_pool(name="e2t", bufs=bp))
        psum_pool = sctx.enter_context(tc.tile_pool(name="ps", bufs=4, space="PSUM"))
        tanh_pool = sctx.enter_context(tc.tile_pool(name="tnh", bufs=3))
        relu_pool = sctx.enter_context(tc.tile_pool(name="rlu", bufs=3))

        def absorb(eng, *prods, hint="absorb"):
            # Park multi-source sem waits on a NOP (which has room for them)
            # so the following fragile-struct instruction needs none.
            no = eng.nop(nofuse=True, hint=hint)
            for p in prods:
                if p is not None:
                    add_dep_helper(
                        no.ins, p.ins if hasattr(p, "ins") else p,
                        reason="wait-absorb",
                    )
            return no

        # a, b -> per-partition scale/bias APs (DMA broadcast via step-0 AP)
        ab_bc = const_pool.tile([P, 2], f32)
        nc.sync.dma_start(ab_bc[:, 0:1], a_in[0:1].broadcast_to((P, 1)))
        nc.sync.dma_start(ab_bc[:, 1:2], b_in[0:1].broadcast_to((P, 1)))
        a_ap = ab_bc[:, 0:1]
        b_ap = ab_bc[:, 1:2]
        hist = {"load": [], "sq": []}

        def norm_cast_transpose(src, n_rows_tiles, n_rows, dst_tiles, s):
            # src: dram [n_rows, d_dim] for sample s; dst_tiles: per-k bf16
            # [P, n_rows] transposed tiles.
            for t in range(n_rows_tiles):
                r0 = t * P
                rows = min(P, n_rows - r0)
                x = nat_pool.tile([P, d_dim], f32, tag="nat")
                nc.sync.dma_start(x[:rows, :], src[r0 : r0 + rows, :])
                sq = sq_pool.tile([P, d_dim], f32, tag="sq")
                n2 = nrm_pool.tile([P, 1], f32, tag="n2")
                nc.scalar.activation(
                    sq[:rows, :],
                    x[:rows, :],
                    mybir.ActivationFunctionType.Square,
                    accum_out=n2[:rows, :],
                )
                nrm = nrm_pool.tile([P, 1], f32, tag="nrm")
                nc.scalar.sqrt(nrm[:rows, :], n2[:rows, :])
                inv = nrm_pool.tile([P, 1], f32, tag="inv")
                nc.vector.reciprocal(inv[:rows, :], nrm[:rows, :])
                xb = bfn_pool.tile([P, d_dim], bf16, tag="xb")
                nc.scalar.mul(xb[:rows, :], x[:rows, :], inv[:rows, :])
                # One xbar transpose for the whole [rows, d_dim] tile.  The
                # 3D out AP lands d = 6*p + k at (partition p, plane k) —
                # an interleaved-d layout; matmul works as long as lhsT and
                # rhs use the same (p, k) -> d mapping.
                nc.sync.dma_start_transpose(
                    dst_tiles[:, :, r0 : r0 + rows], xb[:rows, :]
                )

        for s in range(bp):
            e1t = e1t_pool.tile([P, KT, i_dim], bf16, tag="e1t", name=f"e1t_{s}")
            e2t = e2t_pool.tile([P, KT, j_dim], bf16, tag="e2t", name=f"e2t_{s}")
            norm_cast_transpose(e1[s], IT, i_dim, e1t, s)
            norm_cast_transpose(e2[s], JT, j_dim, e2t, s)

            for it in range(IT):
                ps = psum_pool.tile([P, j_dim], f32, tag="ps")
                for k in range(KT):
                    nc.tensor.matmul(
                        ps[:],
                        e1t[:, k, it * P : (it + 1) * P],
                        e2t[:, k, :],
                        start=(k == 0),
                        stop=(k == KT - 1),
                    )
                # relu(tanh(z)) == tanh(relu(z)); col 0 keeps the raw tanh.
                tr = tanh_pool.tile([P, j_dim], f32, tag="th")
                nc.scalar.activation(
                    tr[:, 1:],
                    ps[:, 1:],
                    mybir.ActivationFunctionType.Relu,
                    bias=b_ap,
                    scale=a_ap,
                )
                rp = relu_pool.tile([P, j_dim], f32, tag="rp")
                nc.scalar.activation(
                    rp[:, 1:], tr[:, 1:], mybir.ActivationFunctionType.Tanh
                )
                nc.scalar.activation(
                    rp[:, 0:1],
                    ps[:, 0:1],
                    mybir.ActivationFunctionType.Tanh,
                    bias=b_ap,
                    scale=a_ap,
                )
                nc.sync.dma_start(dp_r[it * P : (it + 1) * P, s, :], rp[:])

        # ---- DTW DP ----
        sctx.close()

        dpc_pool = ctx.enter_context(tc.tile_pool(name="dpc", bufs=2))
        dpt_pool = ctx.enter_context(tc.tile_pool(name="dpt", bufs=2))
        m_ext = dp_pool.tile([bp, j_dim + 1], f32, tag="m")
        nc.vector.memset(m_ext[:, 0:1], 0.0)
        nc.sync.dma_start(m_ext[:, 1:], dp_r[0, :, :])
        r_last = dp_pool.tile([bp, 1], f32, tag="rl")
        nc.sync.dma_start(r_last[:], dp_r[i_dim - 1 : i_dim, :, j_dim - 1])

        for c0 in range(1, i_dim - 1, chunk):
            ch = min(chunk, i_dim - 1 - c0)
            rch = dpc_pool.tile([bp, chunk, j_dim], f32, tag="rch")
            nc.sync.dma_start(
                rch[:, :ch, :], dp_r[c0 : c0 + ch, :, :].rearrange("i s j -> s i j")
            )
            for i in range(ch):
                tmp = dpt_pool.tile([bp, j_dim], f32, tag="tmp")
                nc.vector.tensor_add(tmp[:], m_ext[:, 0:j_dim], rch[:, i, :])
                nc.vector.tensor_max(m_ext[:, 1:], m_ext[:, 1:], tmp[:])

        ans = dp_pool.tile([bp, 1], f32, tag="ans")
        nc.vector.tensor_add(ans[:], m_ext[:, j_dim - 1 : j_dim], r_last[:])
        nc.vector.tensor_scalar_mul(ans[:], ans[:], 1.0 / j_dim)
        nc.sync.dma_start(out[:], ans[:, 0])

    return nc


_cache = {}


def _get_nc():
    if "nc" not in _cache:
        nc = build_nc()
        nc.finalize()  # run Bacc reg-alloc/DCE before the PJRT compile path
        _cache["nc"] = nc
    return _cache["nc"]


def make_in_maps(emb1, emb2, a, b):
    e1 = np.ascontiguousarray(np.asarray(emb1, np.float32).reshape(N_CORES, BP, I, D))
    e2 = np.ascontiguousarray(np.asarray(emb2, np.float32).reshape(N_CORES, BP, J, D))
    a32 = np.asarray(a, np.float32).reshape(1)
    b32 = np.asarray(b, np.float32).reshape(1)
    return [
        {"e1": e1[c], "e2": e2[c], "a": a32, "b": b32} for c in range(N_CORES)
    ]


def run_spmd(in_maps, **kwargs):
    from concourse.bass_utils import run_bass_kernel_spmd

    return run_bass_kernel_spmd(_get_nc(), in_maps, list(range(N_CORES)), **kwargs)


def kernel(emb1, emb2, a, b):
    res = run_spmd(make_in_maps(emb1, emb2, a, b))
    return np.concatenate(
        [np.asarray(res.results[c]["out"]) for c in range(N_CORES)]
    ).astype(np.float32)


if __name__ == "__main__":
    rng = np.random.default_rng(0)
    inputs = dict(
        emb1=rng.standard_normal((B, I, D), dtype=np.float32),
        emb2=rng.standard_normal((B, J, D), dtype=np.float32),
        a=rng.random((1,), dtype=np.float32),
        b=rng.random((1,), dtype=np.float32),
    )
    out = kernel(**inputs)
    print("out[:4]:", out[:4])
